# revision 20
# baseline (speedup 1.0000x reference)
"""Trainium2 Bass kernel for nn_ChannelAttentionModule (B=8, H=W=128, C=512).

Reference computation (per sample, q = inputs reshaped to [HW, C] = [16384, 512]):
    S = q^T @ q                      # [C, C]
    P = softmax(max_row(S) - S)      # == softmax(-S) row-wise (shift invariant)
    out = gamma * (q @ P) + q

Numerical scheme: q is split as q ~= hi + lo (both fp16, lo^T lo dropped,
~1e-7 relative).  With X = hi^T lo (note lo^T hi == X^T exactly):
    S = hi^T hi + X + X^T = A + A^T
where A = strict_upper(hi^T hi) + 0.5 * blockdiag(hi^T hi) + X is accumulated
in 4 PSUM banks (block-row per bank), and S = A + A^T is assembled once on
the small [512, 512] matrix via fp32 PE transposes.  The value pass uses
M = gamma * softmax(-S) + I so the gamma-scale and residual add ride through
the matmul.

Sharding: data-parallel over batch, one sample per NeuronCore, 8 cores, no
cross-core communication (gamma replicated host-side).

Per-core schedule:
  pass 1: stream 64 x [128, 2, 512] fp32 slabs of q (512KB contiguous DMAs);
          cast to resident fp16 hi (VectorE), lo = q - hi, hi/2 (exact);
          accumulate A on TensorE (fp16, 3328 PE columns per 128-row chunk).
  fixup:  S = A + A^T; row-min + exp with fused row-sum (ScalarE);
          M = gamma*P + I cast to fp16.
  pass 2: per 128-row chunk: transpose resident hi via matmul-vs-identity,
          4 value matmuls accumulate out = q @ M in PSUM, evacuate
          (VectorE/ScalarE alternating), 512KB DMAs back to HBM.
"""

import sys

for _p in ("/opt/trn_rl_repo",):
    if _p not in sys.path:
        sys.path.insert(0, _p)

from contextlib import ExitStack

import numpy as np

import concourse.bass as bass
import concourse.mybir as mybir
import concourse.tile as tile
from concourse import bacc

F32 = mybir.dt.float32
F16 = mybir.dt.float16
F8E4 = mybir.dt.float8e4
F8E5 = mybir.dt.float8e5

P = 128
C = 512
MB = C // P  # 4 c-blocks

B, H, W = 8, 128, 128
N_ROWS_FULL = H * W  # 16384 rows per sample
N_CORES = 8


def build(n_rows=N_ROWS_FULL, s_mode="f8dr", dma_tr=False):
    """Per-core kernel. Inputs: x [n_rows, C] f32, gammab [128,1] f32,
    ident [128,128] f16. Output: out [n_rows, C] f32.

    s_mode:
      "f16hl" - S via fp16 hi/lo split, X = hi^T lo in fp16 (baseline).
      "f8dr"  - X = hi8^T lo8 in fp8-e4m3 DoubleRow matmuls (2 contraction
                rows per cycle) accumulating 4096*X in 4 separate PSUM
                banks; descaled by 2^-12 during the fixup.  lo8 = e4m3(
                4096*(q - hi)) so the product scale is an exact power of 2.
    """
    assert n_rows % 256 == 0
    nsup = n_rows // 256
    nsub = n_rows // 128
    use_lo = s_mode == "f16hl"
    use_f8 = s_mode in ("f8dr", "f8", "f8nomm", "f8tt")
    use_dr = s_mode == "f8dr"
    f8_mm = s_mode in ("f8dr", "f8")
    use_ttr = s_mode != "f8tt"

    nc = bacc.Bacc(trn_type="TRN2", name="chanattn")
    x = nc.dram_tensor("x", [n_rows, C], F32, kind="ExternalInput")
    gb = nc.dram_tensor("gammab", [P, 1], F32, kind="ExternalInput")
    idm = nc.dram_tensor("ident", [P, P], F16, kind="ExternalInput")
    out = nc.dram_tensor("out", [n_rows, C], F32, kind="ExternalOutput")

    # row r = 256*i + 2*p + j: partition p reads 4KB contiguous per super-chunk
    xv = x[:].rearrange("(i p j) c -> i p j c", p=P, j=2)
    ov = out[:].rearrange("(i p j) c -> i p j c", p=P, j=2)

    with tile.TileContext(nc) as tc, ExitStack() as ctx:
        pers = ctx.enter_context(tc.tile_pool(name="pers", bufs=1))
        p_in = ctx.enter_context(tc.tile_pool(name="p_in", bufs=3))
        p_h = ctx.enter_context(tc.tile_pool(name="p_h", bufs=2))
        p_sm = ctx.enter_context(tc.tile_pool(name="p_sm", bufs=2))
        p_qc = ctx.enter_context(tc.tile_pool(name="p_qc", bufs=10))
        p_out = ctx.enter_context(tc.tile_pool(name="p_out", bufs=2))
        ps_a_ctx = ExitStack()
        ps_a = ps_a_ctx.enter_context(tc.tile_pool(name="ps_a", bufs=1, space="PSUM"))
        if use_f8:
            ps_x = ps_a_ctx.enter_context(
                tc.tile_pool(name="ps_x", bufs=1, space="PSUM")
            )

        # resident fp16 hi = round(q), laid out [p, sub, c]
        hi_res = pers.tile([P, nsub, C], F16)
        xf0 = p_in.tile([P, 2, C], F32, tag="xf", name="xf")
        nc.sync.dma_start(xf0[:], xv[0])
        ident16 = pers.tile([P, P], F16)
        nc.sync.dma_start(ident16[:], idm[:])
        gamma_sb = pers.tile([P, 1], F32)
        nc.sync.dma_start(gamma_sb[:], gb[:])

        # A = strict_upper(hh) + 0.5*diag-blocks(hh) [+ X if f16hl], block-row
        # m per bank; in f8dr mode 4096*X accumulates in 4 separate banks.
        a_ps = [ps_a.tile([P, C], F32, tag=f"a{m}", name=f"a{m}") for m in range(MB)]
        if use_f8:
            x_ps = [
                ps_x.tile([P, C], F32, tag=f"x{m}", name=f"x{m}") for m in range(MB)
            ]

        # ---- pass 1 ----
        for i in range(nsup):
            if i == 0:
                xf = xf0
            else:
                xf = p_in.tile([P, 2, C], F32, tag="xf", name="xf")
                nc.sync.dma_start(xf[:], xv[i])
            hi2 = hi_res[:, 2 * i : 2 * i + 2, :]
            nc.vector.tensor_copy(hi2, xf[:])
            xh2 = p_h.tile([P, 2, C], F16, tag="xh2", name="xh2")
            nc.scalar.mul(xh2[:], hi2, 0.5)
            if use_lo:
                xl = p_h.tile([P, 2, C], F16, tag="xl", name="xl")
                nc.vector.tensor_tensor(xl[:], xf[:], hi2, mybir.AluOpType.subtract)
            if use_f8:
                f8dt = F8E4 if f8_mm else F16
                f8scale = 4096.0 if f8_mm else 1.0
                hi8 = p_h.tile([P, 2, C], f8dt, tag="hi8", name="hi8")
                nc.scalar.copy(hi8[:], hi2)
                # lo8 = e4m3(4096 * (q - hi)).  NOTE: tensor_tensor_reduce
                # would fuse this, but it crashes the HW runtime (CoreSim
                # accepts it) - keep it as two DVE ops.
                lo16 = p_h.tile([P, 2, C], F16, tag="lo16", name="lo16")
                nc.vector.tensor_tensor(lo16[:], xf[:], hi2, mybir.AluOpType.subtract)
                lo8 = p_h.tile([P, 2, C], f8dt, tag="lo8", name="lo8")
                nc.vector.tensor_scalar_mul(lo8[:], lo16[:], f8scale)
                if use_dr:
                    for m in range(MB):
                        nc.tensor.matmul(
                            x_ps[m][:],
                            hi8[:, :, m * P : (m + 1) * P],
                            lo8[:],
                            start=(i == 0),
                            stop=(i == nsup - 1),
                            perf_mode=mybir.MatmulPerfMode.DoubleRow,
                            skip_group_check=True,
                        )
                else:
                    for jj in range(2):
                        for m in range(MB):
                            nc.tensor.matmul(
                                x_ps[m][:],
                                hi8[:, jj, m * P : (m + 1) * P],
                                lo8[:, jj, :],
                                start=(i == 0 and jj == 0),
                                stop=(i == nsup - 1 and jj == 1),
                                skip_group_check=True,
                            )
            for j in range(2):
                s = 2 * i + j
                first = s == 0
                last = s == nsub - 1
                hs = hi_res[:, s, :]
                for m in range(MB):
                    lhs = hs[:, m * P : (m + 1) * P]
                    # 0.5 * hh diagonal block. Only THIS matmul at s==0 may
                    # carry start=True: start clears has_written for the
                    # whole bank, so later same-bank groups must rely on the
                    # cleared bits (overwrite-then-set) instead of start.
                    nc.tensor.matmul(
                        a_ps[m][:, m * P : (m + 1) * P],
                        lhs,
                        xh2[:, j, m * P : (m + 1) * P],
                        start=first,
                        stop=last,
                        skip_group_check=True,
                    )
                    # strict-upper hh blocks
                    if m < MB - 1:
                        nc.tensor.matmul(
                            a_ps[m][:, (m + 1) * P :],
                            lhs,
                            hs[:, (m + 1) * P :],
                            start=False,
                            stop=last,
                            skip_group_check=True,
                        )
                    if use_lo:
                        nc.tensor.matmul(
                            a_ps[m][:],
                            lhs,
                            xl[:, j, :],
                            start=False,
                            stop=last,
                            skip_group_check=True,
                        )

        # ---- assemble S = A + A^T in SBUF ----
        # A^T via hi/lo fp16 split + regular matmul-vs-identity transposes
        # (fp32 PE transposes are fused-LDW and can only carry one sync wait,
        # which walrus rejects here).  hi+lo carries ~21 bits of A; the lost
        # precision only affects entries with |A| huge (the diag blocks),
        # which are irrelevant to the row-min softmax.
        s_sb = pers.tile([P, MB, C], F32)
        for m in range(MB):
            if use_f8:
                # A bank m only holds columns [m*P:] (diag-half + strict
                # upper); the strict-lower columns come from X alone.
                nc.scalar.mul(s_sb[:, m, :], x_ps[m][:], 2.0**-12 if f8_mm else 1.0)
                nc.vector.tensor_tensor(
                    s_sb[:, m, m * P :],
                    s_sb[:, m, m * P :],
                    a_ps[m][:, m * P :],
                    mybir.AluOpType.add,
                )
            else:
                nc.vector.tensor_copy(s_sb[:, m, :], a_ps[m][:])
        ps_a_ctx.close()
        ps_t = ctx.enter_context(tc.tile_pool(name="ps_t", bufs=3, space="PSUM"))
        ps_v = ctx.enter_context(tc.tile_pool(name="ps_v", bufs=3, space="PSUM"))
        ah = pers.tile([P, MB, C], F16)
        al = pers.tile([P, MB, C], F16)
        for m in range(MB):
            nc.vector.tensor_copy(ah[:, m, :], s_sb[:, m, :])
            nc.vector.tensor_tensor(
                al[:, m, :], s_sb[:, m, :], ah[:, m, :], mybir.AluOpType.subtract
            )
        # S[m, mp] = A[m, mp] + T(Ah[mp, m]) + T(Al[mp, m]); the hi and lo
        # transposes accumulate in PSUM so one DVE add per block suffices.
        for mp in range(MB):
            for m in range(MB):
                tp = ps_t.tile([P, MB, P], F32, tag="tp", name="tp")
                nc.tensor.matmul(
                    tp[:, 0, :],
                    ah[:, mp, m * P : (m + 1) * P],
                    ident16[:],
                    start=True,
                    stop=False,
                )
                nc.tensor.matmul(
                    tp[:, 0, :],
                    al[:, mp, m * P : (m + 1) * P],
                    ident16[:],
                    start=False,
                    stop=True,
                )
                nc.vector.tensor_tensor(
                    s_sb[:, m, mp * P : (mp + 1) * P],
                    s_sb[:, m, mp * P : (mp + 1) * P],
                    tp[:, 0, :],
                    mybir.AluOpType.add,
                )

        # ---- softmax: M = gamma * softmax(-S) + I (fp16) ----
        mfull = pers.tile([P, MB, C], F16)
        for m in range(MB):
            mn = p_sm.tile([P, 1], F32, tag="mn", name="mn")
            nc.vector.tensor_reduce(
                mn[:], s_sb[:, m, :], axis=mybir.AxisListType.X, op=mybir.AluOpType.min
            )
            e = p_sm.tile([P, C], F32, tag="e", name="e")
            z = p_sm.tile([P, 1], F32, tag="z", name="z")
            nc.scalar.activation(
                e[:],
                s_sb[:, m, :],
                mybir.ActivationFunctionType.Exp,
                bias=mn[:],
                scale=-1.0,
                accum_out=z[:],
            )
            rz = p_sm.tile([P, 1], F32, tag="rz", name="rz")
            nc.vector.reciprocal(rz[:], z[:])
            rzg = p_sm.tile([P, 1], F32, tag="rzg", name="rzg")
            nc.vector.tensor_mul(rzg[:], rz[:], gamma_sb[:])
            nc.vector.tensor_scalar_mul(mfull[:, m, :], e[:], rzg[:])
            nc.vector.tensor_tensor(
                mfull[:, m, m * P : (m + 1) * P],
                mfull[:, m, m * P : (m + 1) * P],
                ident16[:],
                mybir.AluOpType.add,
            )

        # ---- pass 2: out = q @ M (chunk-transpose + 4 accumulating matmuls) ----
        qc_tiles = {}

        def emit_tr(s):
            qc = p_qc.tile([P, MB, P], F16, tag="qc", name="qc")
            if dma_tr:
                for m in range(MB):
                    nc.sync.dma_start_transpose(
                        qc[:, m, :], hi_res[:, s, m * P : (m + 1) * P]
                    )
            else:
                tp = ps_t.tile([P, MB, P], F32, tag="tp", name="tp")
                for m in range(MB):
                    nc.tensor.matmul(
                        tp[:, m, :],
                        hi_res[:, s, m * P : (m + 1) * P],
                        ident16[:],
                        start=True,
                        stop=True,
                    )
                nc.scalar.copy(qc[:], tp[:])
            qc_tiles[s] = qc

        for s0 in range(min(10, nsub)):
            emit_tr(s0)
        for i in range(nsup):
            of = p_out.tile([P, 2, C], F32, tag="of", name="of")
            for j in range(2):
                s = 2 * i + j
                if s + 10 < nsub:
                    emit_tr(s + 10)
                vp = ps_v.tile([P, C], F32, tag="vp", name="vp")
                qc = qc_tiles.pop(s)
                for m in range(MB):
                    nc.tensor.matmul(
                        vp[:],
                        qc[:, m, :],
                        mfull[:, m, :],
                        start=(m == 0),
                        stop=(m == MB - 1),
                    )
                if j == 0:
                    nc.vector.tensor_copy(of[:, j, :], vp[:])
                else:
                    nc.scalar.copy(of[:, j, :], vp[:])
            nc.sync.dma_start(ov[i], of[:])

    nc.compile()
    return nc


def build_v3(n_rows=N_ROWS_FULL, lo_eng="dve", warm=True):
    """v3: all of f8dr plus
      - lo8 = e5m2(q - hi) in ONE DVE (or GpSimd) op: e5m2 normals reach
        6e-5 so no 4096 pre-scale is needed, X accumulates at scale 1.
      - H diag blocks computed at full weight (rhs = hi[:, m*P:], one MM
        per (chunk, m)); the double-count is avoided by transposing only
        the X diag in the fixup.  No 0.5*hi tensor, 6 fewer MMs/chunk.
      - q-chunk transposes moved off TensorE onto the DMA xbar
        (dma_start_transpose SBUF->SBUF, ACT HWDGE queue), split across
        the passes by super-chunk parity to respect fabric bandwidth:
        even super-chunks transpose during pass 1 into resident qc_res;
        odd super-chunks keep fp16 hi resident and transpose JIT in
        pass 2.  Pass 2's PE does nothing but value matmuls.
    """
    assert n_rows % 512 == 0
    nsup = n_rows // 256
    nsub = n_rows // 128

    nc = bacc.Bacc(trn_type="TRN2", name="chanattn3")
    x = nc.dram_tensor("x", [n_rows, C], F32, kind="ExternalInput")
    gb = nc.dram_tensor("gammab", [P, 1], F32, kind="ExternalInput")
    idm = nc.dram_tensor("ident", [P, P], F16, kind="ExternalInput")
    out = nc.dram_tensor("out", [n_rows, C], F32, kind="ExternalOutput")

    xv = x[:].rearrange("(i p j) c -> i p j c", p=P, j=2)
    ov = out[:].rearrange("(i p j) c -> i p j c", p=P, j=2)

    lo_sub = {"dve": None, "gps": None}

    with tile.TileContext(nc) as tc, ExitStack() as ctx:
        pers = ctx.enter_context(tc.tile_pool(name="pers", bufs=1))
        p_in = ctx.enter_context(tc.tile_pool(name="p_in", bufs=2))
        p_hi = ctx.enter_context(tc.tile_pool(name="p_hi", bufs=2))
        p_h = ctx.enter_context(tc.tile_pool(name="p_h", bufs=2))
        p_sm = ctx.enter_context(tc.tile_pool(name="p_sm", bufs=2))
        p_qc = ctx.enter_context(tc.tile_pool(name="p_qc", bufs=3))
        p_out = ctx.enter_context(tc.tile_pool(name="p_out", bufs=2))
        ps_a_ctx = ExitStack()
        ps_a = ps_a_ctx.enter_context(tc.tile_pool(name="ps_a", bufs=1, space="PSUM"))
        ps_x = ps_a_ctx.enter_context(tc.tile_pool(name="ps_x", bufs=1, space="PSUM"))

        # resident transposed q (even super-chunks) / fp16 q (odd ones)
        qc_res = pers.tile([P, MB, nsup, P], F16)
        hi_res = pers.tile([P, nsup // 2, 2, C], F16)

        xf0 = p_in.tile([P, 2, C], F32, tag="xf", name="xf")
        nc.sync.dma_start(xf0[:], xv[0])
        ident16 = pers.tile([P, P], F16)
        nc.sync.dma_start(ident16[:], idm[:])
        gamma_sb = pers.tile([P, 1], F32)
        nc.sync.dma_start(gamma_sb[:], gb[:])

        a_ps = [ps_a.tile([P, C], F32, tag=f"a{m}", name=f"a{m}") for m in range(MB)]
        x_ps = [ps_x.tile([P, C], F32, tag=f"x{m}", name=f"x{m}") for m in range(MB)]

        # ---- pass 1 ----
        for i in range(nsup):
            if i == 0:
                xf = xf0
            else:
                xf = p_in.tile([P, 2, C], F32, tag="xf", name="xf")
                nc.sync.dma_start(xf[:], xv[i])
            if i % 2 == 1:
                hi2 = hi_res[:, i // 2]
            else:
                hi2t = p_hi.tile([P, 2, C], F16, tag="hi2", name="hi2")
                hi2 = hi2t[:]
            nc.vector.tensor_copy(hi2, xf[:])
            if i % 2 == 0:
                for j in range(2):
                    nc.scalar.dma_start_transpose(
                        qc_res[:, :, i + j, :], hi2[:, j, :]
                    )
            hi8 = p_h.tile([P, 2, C], F8E4, tag="hi8", name="hi8")
            nc.scalar.copy(hi8[:], hi2)
            lo8 = p_h.tile([P, 2, C], F8E5, tag="lo8", name="lo8")
            eng = nc.gpsimd if lo_eng == "gps" else nc.vector
            eng.tensor_tensor(lo8[:], xf[:], hi2, mybir.AluOpType.subtract)
            # H: one MM per (chunk, m) covering [m*P:], diag at full weight
            for j in range(2):
                s = 2 * i + j
                for m in range(MB):
                    nc.tensor.matmul(
                        a_ps[m][:, m * P :],
                        hi2[:, j, m * P : (m + 1) * P],
                        hi2[:, j, m * P :],
                        start=(s == 0),
                        stop=(s == nsub - 1),
                        skip_group_check=True,
                    )
            # X = hi8^T lo8, DoubleRow over the 256-row super-chunk
            for m in range(MB):
                nc.tensor.matmul(
                    x_ps[m][:],
                    hi8[:, :, m * P : (m + 1) * P],
                    lo8[:],
                    start=(i == 0),
                    stop=(i == nsup - 1),
                    perf_mode=mybir.MatmulPerfMode.DoubleRow,
                    skip_group_check=True,
                )

        # ---- fixup: s_sb = A(+diag) + X; S = s_sb + s_sb^T (blockwise) ----
        s_sb = pers.tile([P, MB, C], F32)
        xd = pers.tile([P, MB, P], F16)
        for m in range(MB):
            # walrus: only one PSUM input per DVE op - stage X via SBUF
            nc.scalar.copy(s_sb[:, m, :], x_ps[m][:])
            nc.vector.tensor_tensor(
                s_sb[:, m, m * P :],
                s_sb[:, m, m * P :],
                a_ps[m][:, m * P :],
                mybir.AluOpType.add,
            )
            # X diag block snapshot (fp16): S[m,m] needs + X[m,m]^T only
            nc.vector.tensor_copy(xd[:, m, :], x_ps[m][:, m * P : (m + 1) * P])
        ps_a_ctx.close()
        ps_t = ctx.enter_context(tc.tile_pool(name="ps_t", bufs=3, space="PSUM"))
        ps_v = ctx.enter_context(tc.tile_pool(name="ps_v", bufs=3, space="PSUM"))
        ah = pers.tile([P, MB, C], F16)
        al = pers.tile([P, MB, C], F16)
        for m in range(MB):
            nc.vector.tensor_copy(ah[:, m, :], s_sb[:, m, :])
            nc.vector.tensor_tensor(
                al[:, m, :], s_sb[:, m, :], ah[:, m, :], mybir.AluOpType.subtract
            )
        # off-diag: S[m,mp] += T(ah[mp,m]) + T(al[mp,m]); diag: += T(xd[m])
        for mp in range(MB):
            for m in range(MB):
                tp = ps_t.tile([P, P], F32, tag="tp", name="tp")
                if m == mp:
                    nc.tensor.matmul(
                        tp[:], xd[:, m, :], ident16[:], start=True, stop=True
                    )
                else:
                    nc.tensor.matmul(
                        tp[:],
                        ah[:, mp, m * P : (m + 1) * P],
                        ident16[:],
                        start=True,
                        stop=False,
                    )
                    nc.tensor.matmul(
                        tp[:],
                        al[:, mp, m * P : (m + 1) * P],
                        ident16[:],
                        start=False,
                        stop=True,
                    )
                nc.vector.tensor_tensor(
                    s_sb[:, m, mp * P : (mp + 1) * P],
                    s_sb[:, m, mp * P : (mp + 1) * P],
                    tp[:],
                    mybir.AluOpType.add,
                )

        # ---- softmax: M = gamma * softmax(-S) + I (fp16) ----
        mfull = pers.tile([P, MB, C], F16)
        for m in range(MB):
            mn = p_sm.tile([P, 1], F32, tag="mn", name="mn")
            nc.vector.tensor_reduce(
                mn[:], s_sb[:, m, :], axis=mybir.AxisListType.X, op=mybir.AluOpType.min
            )
            e = p_sm.tile([P, C], F32, tag="e", name="e")
            z = p_sm.tile([P, 1], F32, tag="z", name="z")
            nc.scalar.activation(
                e[:],
                s_sb[:, m, :],
                mybir.ActivationFunctionType.Exp,
                bias=mn[:],
                scale=-1.0,
                accum_out=z[:],
            )
            rz = p_sm.tile([P, 1], F32, tag="rz", name="rz")
            nc.vector.reciprocal(rz[:], z[:])
            rzg = p_sm.tile([P, 1], F32, tag="rzg", name="rzg")
            nc.vector.tensor_mul(rzg[:], rz[:], gamma_sb[:])
            nc.vector.tensor_scalar_mul(mfull[:, m, :], e[:], rzg[:])
            nc.vector.tensor_tensor(
                mfull[:, m, m * P : (m + 1) * P],
                mfull[:, m, m * P : (m + 1) * P],
                ident16[:],
                mybir.AluOpType.add,
            )
            if warm:
                # keep HAM un-throttled through the PE-idle softmax window
                wp = ps_t.tile([P, P], F32, tag="tp", name="wp")
                nc.tensor.matmul(wp[:], ident16[:], ident16[:], start=True, stop=True)

        # ---- pass 2: out = q @ M; PE does only the value matmuls ----
        qc_jit = {}

        def emit_jit(io):
            t = p_qc.tile([P, MB, 2, P], F16, tag="qcj", name="qcj")
            for j in range(2):
                nc.scalar.dma_start_transpose(
                    t[:, :, j, :], hi_res[:, io // 2, j, :]
                )
            qc_jit[io] = t

        for io in (1, 3):
            if io < nsup:
                emit_jit(io)
        for i in range(nsup):
            if (i + 4) < nsup and (i + 4) % 2 == 1:
                emit_jit(i + 4)
            of = p_out.tile([P, 2, C], F32, tag="of", name="of")
            for j in range(2):
                if i % 2 == 0:
                    qv = qc_res[:, :, i + j, :]
                else:
                    qv = qc_jit[i][:, :, j, :]
                vp = ps_v.tile([P, C], F32, tag="vp", name="vp")
                for m in range(MB):
                    nc.tensor.matmul(
                        vp[:],
                        qv[:, m, :],
                        mfull[:, m, :],
                        start=(m == 0),
                        stop=(m == MB - 1),
                    )
                if j == 0:
                    nc.vector.tensor_copy(of[:, j, :], vp[:])
                else:
                    nc.scalar.copy(of[:, j, :], vp[:])
            if i % 2 == 1:
                qc_jit.pop(i)
            nc.sync.dma_start(ov[i], of[:])

    nc.compile()
    return nc


def build_v4(
    n_rows=N_ROWS_FULL,
    lo_eng="dve",
    warm=True,
    tr_look=10,
    in_bufs=3,
    h_bufs=2,
    dma_cast=0,
):
    """v4 = v2 structure (resident fp16 hi, PE chunk-transposes in pass 2)
    plus the v3 wins that survived measurement:
      - lo8 = e5m2(q - hi) in ONE DVE op (X scale 1)  [pass-1 DVE was the
        bottleneck: fp32-input tensor_tensor costs ~1.2us/sc, so only one]
      - H diag at full weight, one MM per (chunk, m); X diag transposed
        in the fixup instead (no 0.5*hi tensor, 6 fewer MMs/chunk)
      - X via fp8 DoubleRow in separate PSUM banks
      - dummy PE matmuls through the softmax window to hold HAM at 8/8
    DMA xbar transposes measured 1.25us per chunk (descriptor-bound) -
    not used."""
    assert n_rows % 512 == 0
    nsup = n_rows // 256
    nsub = n_rows // 128

    nc = bacc.Bacc(trn_type="TRN2", name="chanattn4")
    x = nc.dram_tensor("x", [n_rows, C], F32, kind="ExternalInput")
    gb = nc.dram_tensor("gammab", [P, 1], F32, kind="ExternalInput")
    idm = nc.dram_tensor("ident", [P, P], F16, kind="ExternalInput")
    out = nc.dram_tensor("out", [n_rows, C], F32, kind="ExternalOutput")

    xv = x[:].rearrange("(i p j) c -> i p j c", p=P, j=2)
    ov = out[:].rearrange("(i p j) c -> i p j c", p=P, j=2)

    with tile.TileContext(nc) as tc, ExitStack() as ctx:
        pers = ctx.enter_context(tc.tile_pool(name="pers", bufs=1))
        p_in = ctx.enter_context(tc.tile_pool(name="p_in", bufs=in_bufs))
        p_h = ctx.enter_context(tc.tile_pool(name="p_h", bufs=h_bufs))
        p_sm = ctx.enter_context(tc.tile_pool(name="p_sm", bufs=2))
        p_qc = ctx.enter_context(tc.tile_pool(name="p_qc", bufs=tr_look))
        p_out = ctx.enter_context(tc.tile_pool(name="p_out", bufs=2))
        ps_a_ctx = ExitStack()
        ps_a = ps_a_ctx.enter_context(tc.tile_pool(name="ps_a", bufs=1, space="PSUM"))
        ps_x = ps_a_ctx.enter_context(tc.tile_pool(name="ps_x", bufs=1, space="PSUM"))

        hi_res = pers.tile([P, nsub, C], F16)
        xf0 = p_in.tile([P, 2, C], F32, tag="xf", name="xf")
        nc.sync.dma_start(xf0[:], xv[0])
        ident16 = pers.tile([P, P], F16)
        nc.sync.dma_start(ident16[:], idm[:])
        gamma_sb = pers.tile([P, 1], F32)
        nc.sync.dma_start(gamma_sb[:], gb[:])

        a_ps = [ps_a.tile([P, C], F32, tag=f"a{m}", name=f"a{m}") for m in range(MB)]
        x_ps = [ps_x.tile([P, C], F32, tag=f"x{m}", name=f"x{m}") for m in range(MB)]

        # ---- pass 1 ----
        for i in range(nsup):
            if i == 0:
                xf = xf0
            else:
                xf = p_in.tile([P, 2, C], F32, tag="xf", name="xf")
                nc.sync.dma_start(xf[:], xv[i])
            hi2 = hi_res[:, 2 * i : 2 * i + 2, :]
            if dma_cast:
                # SWDGE casting DMA (SBUF->SBUF) frees the DVE of the cast
                nc.gpsimd.dma_start(hi2, xf[:])
            else:
                nc.vector.tensor_copy(hi2, xf[:])
            hi8 = p_h.tile([P, 2, C], F8E4, tag="hi8", name="hi8")
            nc.scalar.copy(hi8[:], hi2)
            lo8 = p_h.tile([P, 2, C], F8E5, tag="lo8", name="lo8")
            eng = nc.gpsimd if lo_eng == "gps" else nc.vector
            eng.tensor_tensor(lo8[:], xf[:], hi2, mybir.AluOpType.subtract)
            for j in range(2):
                s = 2 * i + j
                for m in range(MB):
                    nc.tensor.matmul(
                        a_ps[m][:, m * P :],
                        hi2[:, j, m * P : (m + 1) * P],
                        hi2[:, j, m * P :],
                        start=(s == 0),
                        stop=(s == nsub - 1),
                        skip_group_check=True,
                    )
            for m in range(MB):
                nc.tensor.matmul(
                    x_ps[m][:],
                    hi8[:, :, m * P : (m + 1) * P],
                    lo8[:],
                    start=(i == 0),
                    stop=(i == nsup - 1),
                    perf_mode=mybir.MatmulPerfMode.DoubleRow,
                    skip_group_check=True,
                )

        # ---- fixup: s_sb = X + A (cols >= m*P); S = s_sb + s_sb^T ----
        s_sb = pers.tile([P, MB, C], F32)
        xd = pers.tile([P, MB, P], F16)
        for m in range(MB):
            nc.scalar.copy(s_sb[:, m, :], x_ps[m][:])
            nc.vector.tensor_tensor(
                s_sb[:, m, m * P :],
                s_sb[:, m, m * P :],
                a_ps[m][:, m * P :],
                mybir.AluOpType.add,
            )
            nc.vector.tensor_copy(xd[:, m, :], x_ps[m][:, m * P : (m + 1) * P])
        ps_a_ctx.close()
        ps_t = ctx.enter_context(tc.tile_pool(name="ps_t", bufs=2, space="PSUM"))
        ps_t4 = ctx.enter_context(tc.tile_pool(name="ps_t4", bufs=3, space="PSUM"))
        ps_v = ctx.enter_context(tc.tile_pool(name="ps_v", bufs=3, space="PSUM"))
        ah = pers.tile([P, MB, C], F16)
        al = pers.tile([P, MB, C], F16)
        for m in range(MB):
            nc.vector.tensor_copy(ah[:, m, :], s_sb[:, m, :])
            nc.vector.tensor_tensor(
                al[:, m, :], s_sb[:, m, :], ah[:, m, :], mybir.AluOpType.subtract
            )
        for mp in range(MB):
            for m in range(MB):
                tp = ps_t.tile([P, P], F32, tag="tp", name="tp")
                if m == mp:
                    nc.tensor.matmul(
                        tp[:], xd[:, m, :], ident16[:], start=True, stop=True
                    )
                else:
                    nc.tensor.matmul(
                        tp[:],
                        ah[:, mp, m * P : (m + 1) * P],
                        ident16[:],
                        start=True,
                        stop=False,
                    )
                    nc.tensor.matmul(
                        tp[:],
                        al[:, mp, m * P : (m + 1) * P],
                        ident16[:],
                        start=False,
                        stop=True,
                    )
                nc.vector.tensor_tensor(
                    s_sb[:, m, mp * P : (mp + 1) * P],
                    s_sb[:, m, mp * P : (mp + 1) * P],
                    tp[:],
                    mybir.AluOpType.add,
                )

        # ---- softmax: M = gamma * softmax(-S) + I (fp16) ----
        mfull = pers.tile([P, MB, C], F16)
        for m in range(MB):
            mn = p_sm.tile([P, 1], F32, tag="mn", name="mn")
            nc.vector.tensor_reduce(
                mn[:], s_sb[:, m, :], axis=mybir.AxisListType.X, op=mybir.AluOpType.min
            )
            e = p_sm.tile([P, C], F32, tag="e", name="e")
            z = p_sm.tile([P, 1], F32, tag="z", name="z")
            nc.scalar.activation(
                e[:],
                s_sb[:, m, :],
                mybir.ActivationFunctionType.Exp,
                bias=mn[:],
                scale=-1.0,
                accum_out=z[:],
            )
            rz = p_sm.tile([P, 1], F32, tag="rz", name="rz")
            nc.vector.reciprocal(rz[:], z[:])
            rzg = p_sm.tile([P, 1], F32, tag="rzg", name="rzg")
            nc.vector.tensor_mul(rzg[:], rz[:], gamma_sb[:])
            nc.vector.tensor_scalar_mul(mfull[:, m, :], e[:], rzg[:])
            nc.vector.tensor_tensor(
                mfull[:, m, m * P : (m + 1) * P],
                mfull[:, m, m * P : (m + 1) * P],
                ident16[:],
                mybir.AluOpType.add,
            )
            if warm:
                wp = ps_t.tile([P, P], F32, tag="tp", name="wp")
                nc.tensor.matmul(wp[:], ident16[:], ident16[:], start=True, stop=True)

        # ---- pass 2: PE chunk-transpose + value matmuls ----
        qc_tiles = {}

        def emit_tr(s):
            qc = p_qc.tile([P, MB, P], F16, tag="qc", name="qc")
            tp = ps_t4.tile([P, MB, P], F32, tag="tp4", name="tp4")
            for m in range(MB):
                nc.tensor.matmul(
                    tp[:, m, :],
                    hi_res[:, s, m * P : (m + 1) * P],
                    ident16[:],
                    start=True,
                    stop=True,
                )
            nc.scalar.copy(qc[:], tp[:])
            qc_tiles[s] = qc

        for s0 in range(min(tr_look, nsub)):
            emit_tr(s0)
        for i in range(nsup):
            of = p_out.tile([P, 2, C], F32, tag="of", name="of")
            for j in range(2):
                s = 2 * i + j
                if s + tr_look < nsub:
                    emit_tr(s + tr_look)
                vp = ps_v.tile([P, C], F32, tag="vp", name="vp")
                qc = qc_tiles.pop(s)
                for m in range(MB):
                    nc.tensor.matmul(
                        vp[:],
                        qc[:, m, :],
                        mfull[:, m, :],
                        start=(m == 0),
                        stop=(m == MB - 1),
                    )
                if j == 0:
                    nc.vector.tensor_copy(of[:, j, :], vp[:])
                else:
                    nc.scalar.copy(of[:, j, :], vp[:])
            nc.sync.dma_start(ov[i], of[:])

    nc.compile()
    return nc


def build_v5(
    n_rows=N_ROWS_FULL,
    tr_look=16,
    in_bufs=3,
    h_bufs=3,
    warm=True,
    nwarm=14,
    split0=True,
):
    """v5: single fp16 matmul family for S.

    h2 = fp16(0.5*x) (ScalarE, exact: power-of-2 scale commutes with rounding)
    r  = fp16(x - h2) = 0.5*h + l + eps    (DVE; x - h2 exact in fp32)
    B  = h2^T r accumulated in 4 PSUM banks; S = 2*(B + B^T)
       = hh + h^T l + l^T h + O(2^-13) -- same accuracy class as v4's fp8 X.
    The x2 rides the fixup PSUM->SBUF copy (scalar.mul 2.0); gammab input
    must carry 2*gamma; the +I residual uses a 2*I tile (idg).
    Removes: DVE fp16 cast, fp8 hi8/lo8 casts, fp8 DR matmuls, xd diag
    special-case.  Pass-1 PE: 8x N=512 MM per super-chunk (1727 ns) vs
    DMA 1430 ns.  Startup: memset warm-up matmuls keep HAM at 8/8 through
    the first DMA wait; first super-chunk DMA split per-j halves.
    Pass 2: per chunk one ~650ns PSUM evac op on each of ScalarE/DVE
    (qc-copy and out-evac alternate engines by chunk parity).
    """
    assert n_rows % 512 == 0
    nsup = n_rows // 256
    nsub = n_rows // 128

    nc = bacc.Bacc(trn_type="TRN2", name="chanattn5")
    x = nc.dram_tensor("x", [n_rows, C], F32, kind="ExternalInput")
    gb = nc.dram_tensor("gammab", [P, 1], F32, kind="ExternalInput")  # = 2*gamma
    idm = nc.dram_tensor("ident", [P, P], F16, kind="ExternalInput")
    out = nc.dram_tensor("out", [n_rows, C], F32, kind="ExternalOutput")

    xv = x[:].rearrange("(i p j) c -> i p j c", p=P, j=2)
    ov = out[:].rearrange("(i p j) c -> i p j c", p=P, j=2)

    with tile.TileContext(nc) as tc, ExitStack() as ctx:
        pers = ctx.enter_context(tc.tile_pool(name="pers", bufs=1))
        p_in = ctx.enter_context(tc.tile_pool(name="p_in", bufs=in_bufs))
        p_h = ctx.enter_context(tc.tile_pool(name="p_h", bufs=h_bufs))
        p_sm = ctx.enter_context(tc.tile_pool(name="p_sm", bufs=2))
        p_qc = ctx.enter_context(tc.tile_pool(name="p_qc", bufs=tr_look))
        p_out = ctx.enter_context(tc.tile_pool(name="p_out", bufs=2))
        ps_a_ctx = ExitStack()
        ps_a = ps_a_ctx.enter_context(tc.tile_pool(name="ps_a", bufs=1, space="PSUM"))
        ps_w = ps_a_ctx.enter_context(tc.tile_pool(name="ps_w", bufs=1, space="PSUM"))

        hi_res = pers.tile([P, nsub, C], F16)  # holds h2 = 0.5*h
        ident16 = pers.tile([P, P], F16)
        nc.sync.dma_start(ident16[:], idm[:])
        gamma_sb = pers.tile([P, 1], F32)
        nc.sync.dma_start(gamma_sb[:], gb[:])

        # HAM warm-up: zero tile, dummy matmuls run during the first DMA wait
        if nwarm:
            wt = pers.tile([P, C], F16)
            nc.vector.memset(wt[:], 0)
            wps = ps_w.tile([P, C], F32, tag="wps", name="wps")
            for _ in range(nwarm):
                nc.tensor.matmul(
                    wps[:], wt[:, :P], wt[:], start=True, stop=True,
                    skip_group_check=True,
                )

        a_ps = [ps_a.tile([P, C], F32, tag=f"a{m}", name=f"a{m}") for m in range(MB)]

        # ---- pass 1: B = h2^T r ----
        for i in range(nsup):
            xf = p_in.tile([P, 2, C], F32, tag="xf", name="xf")
            if i == 0 and split0:
                # split halves so the j=0 chain starts ~0.7us earlier
                nc.sync.dma_start(xf[:, 0, :], xv[0][:, 0, :])
                nc.sync.dma_start(xf[:, 1, :], xv[0][:, 1, :])
            else:
                nc.sync.dma_start(xf[:], xv[i])
            r = p_h.tile([P, 2, C], F16, tag="r", name="r")
            for j in range(2):
                s = 2 * i + j
                h2j = hi_res[:, s, :]
                nc.scalar.mul(h2j, xf[:, j, :], 0.5)
                nc.vector.tensor_tensor(
                    r[:, j, :], xf[:, j, :], h2j, mybir.AluOpType.subtract
                )
                for m in range(MB):
                    nc.tensor.matmul(
                        a_ps[m][:],
                        hi_res[:, s, m * P : (m + 1) * P],
                        r[:, j, :],
                        start=(s == 0),
                        stop=(s == nsub - 1),
                        skip_group_check=True,
                    )

        # ---- fixup: s_sb = 2*B; S = s_sb + s_sb^T (blockwise PE transposes) ----
        s_sb = pers.tile([P, MB, C], F32)
        for m in range(MB):
            nc.scalar.mul(s_sb[:, m, :], a_ps[m][:], 2.0)
        ps_a_ctx.close()
        ps_t = ctx.enter_context(tc.tile_pool(name="ps_t", bufs=2, space="PSUM"))
        ps_t4 = ctx.enter_context(tc.tile_pool(name="ps_t4", bufs=3, space="PSUM"))
        ps_v = ctx.enter_context(tc.tile_pool(name="ps_v", bufs=3, space="PSUM"))
        ah = pers.tile([P, MB, C], F16)
        al = pers.tile([P, MB, C], F16)
        for m in range(MB):
            nc.vector.tensor_copy(ah[:, m, :], s_sb[:, m, :])
            nc.vector.tensor_tensor(
                al[:, m, :], s_sb[:, m, :], ah[:, m, :], mybir.AluOpType.subtract
            )
        for mp in range(MB):
            for m in range(MB):
                tp = ps_t.tile([P, P], F32, tag="tp", name="tp")
                nc.tensor.matmul(
                    tp[:],
                    ah[:, mp, m * P : (m + 1) * P],
                    ident16[:],
                    start=True,
                    stop=False,
                )
                nc.tensor.matmul(
                    tp[:],
                    al[:, mp, m * P : (m + 1) * P],
                    ident16[:],
                    start=False,
                    stop=True,
                )
                nc.vector.tensor_tensor(
                    s_sb[:, m, mp * P : (mp + 1) * P],
                    s_sb[:, m, mp * P : (mp + 1) * P],
                    tp[:],
                    mybir.AluOpType.add,
                )

        # ---- softmax: M = 2*gamma*softmax(-S) + 2*I (fp16) ----
        idg = pers.tile([P, P], F16)
        nc.scalar.mul(idg[:], ident16[:], 2.0)
        mfull = pers.tile([P, MB, C], F16)
        for m in range(MB):
            mn = p_sm.tile([P, 1], F32, tag="mn", name="mn")
            nc.vector.tensor_reduce(
                mn[:], s_sb[:, m, :], axis=mybir.AxisListType.X, op=mybir.AluOpType.min
            )
            e = p_sm.tile([P, C], F32, tag="e", name="e")
            z = p_sm.tile([P, 1], F32, tag="z", name="z")
            nc.scalar.activation(
                e[:],
                s_sb[:, m, :],
                mybir.ActivationFunctionType.Exp,
                bias=mn[:],
                scale=-1.0,
                accum_out=z[:],
            )
            rz = p_sm.tile([P, 1], F32, tag="rz", name="rz")
            nc.vector.reciprocal(rz[:], z[:])
            rzg = p_sm.tile([P, 1], F32, tag="rzg", name="rzg")
            nc.vector.tensor_mul(rzg[:], rz[:], gamma_sb[:])
            nc.vector.tensor_scalar_mul(mfull[:, m, :], e[:], rzg[:])
            nc.vector.tensor_tensor(
                mfull[:, m, m * P : (m + 1) * P],
                mfull[:, m, m * P : (m + 1) * P],
                idg[:],
                mybir.AluOpType.add,
            )
            if warm:
                wp = ps_t.tile([P, P], F32, tag="tp", name="wp")
                nc.tensor.matmul(wp[:], ident16[:], ident16[:], start=True, stop=True)

        # ---- pass 2: PE chunk-transpose + value matmuls ----
        qc_tiles = {}

        def emit_tr(s):
            qc = p_qc.tile([P, MB, P], F16, tag="qc", name="qc")
            tp = ps_t4.tile([P, MB, P], F32, tag="tp4", name="tp4")
            for m in range(MB):
                nc.tensor.matmul(
                    tp[:, m, :],
                    hi_res[:, s, m * P : (m + 1) * P],
                    ident16[:],
                    start=True,
                    stop=True,
                )
            if s % 2 == 0:
                nc.scalar.copy(qc[:], tp[:])
            else:
                nc.vector.tensor_copy(qc[:], tp[:])
            qc_tiles[s] = qc

        for s0 in range(min(tr_look, nsub)):
            emit_tr(s0)
        for i in range(nsup):
            of = p_out.tile([P, 2, C], F32, tag="of", name="of")
            for j in range(2):
                s = 2 * i + j
                if s + tr_look < nsub:
                    emit_tr(s + tr_look)
                vp = ps_v.tile([P, C], F32, tag="vp", name="vp")
                qc = qc_tiles.pop(s)
                for m in range(MB):
                    nc.tensor.matmul(
                        vp[:],
                        qc[:, m, :],
                        mfull[:, m, :],
                        start=(m == 0),
                        stop=(m == MB - 1),
                    )
                # evac engine opposite to this chunk's qc-copy engine
                if s % 2 == 1:
                    nc.scalar.copy(of[:, j, :], vp[:])
                else:
                    nc.vector.tensor_copy(of[:, j, :], vp[:])
            nc.sync.dma_start(ov[i], of[:])

    nc.compile()
    return nc


def build_v7(
    n_rows=N_ROWS_FULL,
    tr_look=10,
    in_bufs=3,
    h_bufs=3,
    nwarm=14,
    n_early=6,
    ps_t4_bufs=3,
    ps_v_bufs=3,
):
    """v7 = v6 + IO batched 2 super-chunks per DMA (1MB transfers: halves
    the DMA count -> fewer sem-rollover reset barriers and issue ops),
    final output DMA split in half (shorter tail), and the warm-up PSUM
    bank recycled right after pass 1 into an early-transpose bank so
    n_early pass-2 chunk transposes run during the fixup/softmax window
    (fills the pass1->pass2 PE gap, keeps HAM warm).
    """
    assert n_rows % 1024 == 0
    nsub = n_rows // 128
    nk = n_rows // 512  # 2 super-chunks per batch

    nc = bacc.Bacc(trn_type="TRN2", name="chanattn7")
    x = nc.dram_tensor("x", [n_rows, C], F32, kind="ExternalInput")
    gb = nc.dram_tensor("gammab", [P, 1], F32, kind="ExternalInput")  # = 2*gamma
    idm = nc.dram_tensor("ident", [P, P], F16, kind="ExternalInput")
    out = nc.dram_tensor("out", [n_rows, C], F32, kind="ExternalOutput")

    xw = x[:].rearrange("(k u p j) c -> k p u j c", p=P, j=2, u=2)
    ow = out[:].rearrange("(k u p j) c -> k p u j c", p=P, j=2, u=2)

    with tile.TileContext(nc) as tc, ExitStack() as ctx:
        pers = ctx.enter_context(tc.tile_pool(name="pers", bufs=1))
        p_in = ctx.enter_context(tc.tile_pool(name="p_in", bufs=in_bufs))
        p_h = ctx.enter_context(tc.tile_pool(name="p_h", bufs=h_bufs))
        p_sm = ctx.enter_context(tc.tile_pool(name="p_sm", bufs=2))
        p_qc = ctx.enter_context(tc.tile_pool(name="p_qc", bufs=tr_look))
        p_out = ctx.enter_context(tc.tile_pool(name="p_out", bufs=2))
        ps_a_ctx = ExitStack()
        ps_a = ps_a_ctx.enter_context(tc.tile_pool(name="ps_a", bufs=1, space="PSUM"))
        ps_w_ctx = ExitStack()
        ps_w = ps_w_ctx.enter_context(tc.tile_pool(name="ps_w", bufs=1, space="PSUM"))

        hi_res = pers.tile([P, nsub, C], F16)  # holds h2 = 0.5*h
        ident16 = pers.tile([P, P], F16)
        nc.sync.dma_start(ident16[:], idm[:])
        gamma_sb = pers.tile([P, 1], F32)
        nc.sync.dma_start(gamma_sb[:], gb[:])

        if nwarm:
            wt = pers.tile([P, C], F16)
            nc.vector.memset(wt[:], 0)
            wps = ps_w.tile([P, C], F32, tag="wps", name="wps")
            for _ in range(nwarm):
                nc.tensor.matmul(
                    wps[:], wt[:, :P], wt[:], start=True, stop=True,
                    skip_group_check=True,
                )

        a_ps = [ps_a.tile([P, C], F32, tag=f"a{m}", name=f"a{m}") for m in range(MB)]

        # ---- pass 1: B = h2^T r ----
        for k in range(nk):
            xf = p_in.tile([P, 2, 2, C], F32, tag="xf", name="xf")
            if k == 0:
                for u in range(2):
                    for j in range(2):
                        nc.sync.dma_start(xf[:, u, j, :], xw[0][:, u, j, :])
            else:
                nc.sync.dma_start(xf[:], xw[k])
            r = p_h.tile([P, 2, 2, C], F16, tag="r", name="r")
            for u in range(2):
                for j in range(2):
                    s = 4 * k + 2 * u + j
                    h2j = hi_res[:, s, :]
                    nc.scalar.mul(h2j, xf[:, u, j, :], 0.5)
                    nc.vector.tensor_tensor(
                        r[:, u, j, :], xf[:, u, j, :], h2j, mybir.AluOpType.subtract
                    )
                    for m in range(MB):
                        nc.tensor.matmul(
                            a_ps[m][:],
                            hi_res[:, s, m * P : (m + 1) * P],
                            r[:, u, j, :],
                            start=(s == 0),
                            stop=(s == nsub - 1),
                            skip_group_check=True,
                        )
        ps_w_ctx.close()
        ps_e_ctx = ExitStack()
        ps_e = ps_e_ctx.enter_context(tc.tile_pool(name="ps_e", bufs=1, space="PSUM"))

        # early pass-2 transposes through the freed warm bank, overlapping
        # the fixup/softmax window below (scheduler orders by deps)
        qc_tiles = {}

        def emit_tr(s, pool, bank_pool):
            qc = p_qc.tile([P, MB, P], F16, tag="qc", name="qc")
            tp = bank_pool.tile([P, MB, P], F32, tag="tpe", name="tpe")
            for m in range(MB):
                nc.tensor.matmul(
                    tp[:, m, :],
                    hi_res[:, s, m * P : (m + 1) * P],
                    ident16[:],
                    start=True,
                    stop=True,
                )
            if s % 2 == 0:
                nc.scalar.copy(qc[:], tp[:])
            else:
                nc.vector.tensor_copy(qc[:], tp[:])
            qc_tiles[s] = qc

        for s0 in range(min(n_early, nsub)):
            emit_tr(s0, p_qc, ps_e)

        # ---- fixup in PSUM: a_ps[m] <- B[m] + sum_mp T((ah+al)[mp, m]) ----
        ah = pers.tile([P, MB, C], F16)
        al = pers.tile([P, MB, C], F16)
        for m in range(MB):
            nc.scalar.copy(ah[:, m, :], a_ps[m][:])
            nc.vector.tensor_tensor(
                al[:, m, :], a_ps[m][:], ah[:, m, :], mybir.AluOpType.subtract
            )
        for m in range(MB):
            for mp in range(MB):
                nc.tensor.matmul(
                    a_ps[m][:, mp * P : (mp + 1) * P],
                    ah[:, mp, m * P : (m + 1) * P],
                    ident16[:],
                    start=False,
                    stop=False,
                    skip_group_check=True,
                )
                nc.tensor.matmul(
                    a_ps[m][:, mp * P : (mp + 1) * P],
                    al[:, mp, m * P : (m + 1) * P],
                    ident16[:],
                    start=False,
                    stop=(mp == MB - 1),
                    skip_group_check=True,
                )

        # ---- softmax from PSUM: mfull = 2*gamma*softmax(-S) + 2*I (fp16) ----
        idg = pers.tile([P, P], F16)
        nc.scalar.mul(idg[:], ident16[:], 2.0)
        mfull = pers.tile([P, MB, C], F16)
        for m in range(MB):
            mn = p_sm.tile([P, 1], F32, tag="mn", name="mn")
            nc.vector.tensor_reduce(
                mn[:], a_ps[m][:], axis=mybir.AxisListType.X, op=mybir.AluOpType.min
            )
            mn2 = p_sm.tile([P, 1], F32, tag="mn2", name="mn2")
            nc.vector.tensor_scalar_mul(mn2[:], mn[:], 2.0)
            e = p_sm.tile([P, C], F32, tag="e", name="e")
            z = p_sm.tile([P, 1], F32, tag="z", name="z")
            nc.scalar.activation(
                e[:],
                a_ps[m][:],
                mybir.ActivationFunctionType.Exp,
                bias=mn2[:],
                scale=-2.0,
                accum_out=z[:],
            )
            rz = p_sm.tile([P, 1], F32, tag="rz", name="rz")
            nc.vector.reciprocal(rz[:], z[:])
            rzg = p_sm.tile([P, 1], F32, tag="rzg", name="rzg")
            nc.vector.tensor_mul(rzg[:], rz[:], gamma_sb[:])
            nc.scalar.mul(mfull[:, m, :], e[:], rzg[:])
            nc.vector.tensor_tensor(
                mfull[:, m, m * P : (m + 1) * P],
                mfull[:, m, m * P : (m + 1) * P],
                idg[:],
                mybir.AluOpType.add,
            )
        ps_e_ctx.close()
        ps_a_ctx.close()
        ps_t4 = ctx.enter_context(
            tc.tile_pool(name="ps_t4", bufs=ps_t4_bufs, space="PSUM")
        )
        ps_v = ctx.enter_context(tc.tile_pool(name="ps_v", bufs=ps_v_bufs, space="PSUM"))

        # ---- pass 2: PE chunk-transpose + value matmuls ----
        for s0 in range(n_early, min(tr_look, nsub)):
            emit_tr(s0, p_qc, ps_t4)
        for k in range(nk):
            of = p_out.tile([P, 2, 2, C], F32, tag="of", name="of")
            for u in range(2):
                for j in range(2):
                    s = 4 * k + 2 * u + j
                    if s + tr_look < nsub:
                        emit_tr(s + tr_look, p_qc, ps_t4)
                    vp = ps_v.tile([P, C], F32, tag="vp", name="vp")
                    qc = qc_tiles.pop(s)
                    for m in range(MB):
                        nc.tensor.matmul(
                            vp[:],
                            qc[:, m, :],
                            mfull[:, m, :],
                            start=(m == 0),
                            stop=(m == MB - 1),
                        )
                    if s % 2 == 1:
                        nc.scalar.copy(of[:, u, j, :], vp[:])
                    else:
                        nc.vector.tensor_copy(of[:, u, j, :], vp[:])
            if k == nk - 1:
                for u in range(2):
                    nc.sync.dma_start(ow[k][:, u, :, :], of[:, u, :, :])
            else:
                nc.sync.dma_start(ow[k], of[:])

    nc.compile()
    return nc


def build_v8(
    n_rows=N_ROWS_FULL,
    tr_look=10,
    in_bufs=3,
    h_bufs=3,
    nwarm=12,
    n_early=10,
    ps_t4_bufs=4,
    ps_v_bufs=3,
    memset_warm=True,
):
    """v7 = v6 + IO batched 2 super-chunks per DMA (1MB transfers: halves
    the DMA count -> fewer sem-rollover reset barriers and issue ops),
    final output DMA split in half (shorter tail), and the warm-up PSUM
    bank recycled right after pass 1 into an early-transpose bank so
    n_early pass-2 chunk transposes run during the fixup/softmax window
    (fills the pass1->pass2 PE gap, keeps HAM warm).
    """
    assert n_rows % 1024 == 0
    nsub = n_rows // 128
    nk = n_rows // 512  # 2 super-chunks per batch

    nc = bacc.Bacc(trn_type="TRN2", name="chanattn8")
    x = nc.dram_tensor("x", [n_rows, C], F32, kind="ExternalInput")
    gb = nc.dram_tensor("gammab", [P, 1], F32, kind="ExternalInput")  # = 2*gamma
    idm = nc.dram_tensor("ident", [P, P], F16, kind="ExternalInput")
    out = nc.dram_tensor("out", [n_rows, C], F32, kind="ExternalOutput")

    xw = x[:].rearrange("(k u p j) c -> k p u j c", p=P, j=2, u=2)
    ow = out[:].rearrange("(k u p j) c -> k p u j c", p=P, j=2, u=2)

    with tile.TileContext(nc) as tc, ExitStack() as ctx:
        pers = ctx.enter_context(tc.tile_pool(name="pers", bufs=1))
        p_in = ctx.enter_context(tc.tile_pool(name="p_in", bufs=in_bufs))
        p_h = ctx.enter_context(tc.tile_pool(name="p_h", bufs=h_bufs))
        p_sm = ctx.enter_context(tc.tile_pool(name="p_sm", bufs=2))
        p_qc = ctx.enter_context(tc.tile_pool(name="p_qc", bufs=tr_look))
        p_out = ctx.enter_context(tc.tile_pool(name="p_out", bufs=2))
        ps_v = ctx.enter_context(
            tc.tile_pool(name="ps_v", bufs=ps_v_bufs, space="PSUM")
        )
        ps_a_ctx = ExitStack()
        ps_a = ps_a_ctx.enter_context(tc.tile_pool(name="ps_a", bufs=1, space="PSUM"))
        ps_w_ctx = ExitStack()
        ps_w = ps_w_ctx.enter_context(tc.tile_pool(name="ps_w", bufs=1, space="PSUM"))

        hi_res = pers.tile([P, nsub, C], F16)  # holds h2 = 0.5*h
        ident16 = pers.tile([P, P], F16)
        gamma_sb = pers.tile([P, 1], F32)

        if nwarm:
            wt = pers.tile([P, C], F16)
            if memset_warm:
                nc.vector.memset(wt[:], 0)
            wps = ps_w.tile([P, C], F32, tag="wps", name="wps")
            for _ in range(nwarm):
                nc.tensor.matmul(
                    wps[:], wt[:, :P], wt[:], start=True, stop=True,
                    skip_group_check=True,
                )

        a_ps = [ps_a.tile([P, C], F32, tag=f"a{m}", name=f"a{m}") for m in range(MB)]

        # ---- pass 1: B = h2^T r ----
        for k in range(nk):
            xf = p_in.tile([P, 2, 2, C], F32, tag="xf", name="xf")
            if k == 0:
                for u in range(2):
                    for j in range(2):
                        nc.sync.dma_start(xf[:, u, j, :], xw[0][:, u, j, :])
            else:
                for u in range(2):
                    nc.sync.dma_start(xf[:, u, :, :], xw[k][:, u, :, :])
            if k == 0:
                # issued after the first input batch so x data lands first
                nc.sync.dma_start(ident16[:], idm[:])
                nc.sync.dma_start(gamma_sb[:], gb[:])
            r = p_h.tile([P, 2, 2, C], F16, tag="r", name="r")
            for u in range(2):
                for j in range(2):
                    s = 4 * k + 2 * u + j
                    h2j = hi_res[:, s, :]
                    nc.scalar.mul(h2j, xf[:, u, j, :], 0.5)
                    nc.vector.tensor_tensor(
                        r[:, u, j, :], xf[:, u, j, :], h2j, mybir.AluOpType.subtract
                    )
                    for m in range(MB):
                        nc.tensor.matmul(
                            a_ps[m][:],
                            hi_res[:, s, m * P : (m + 1) * P],
                            r[:, u, j, :],
                            start=(s == 0),
                            stop=(s == nsub - 1),
                            skip_group_check=True,
                        )
        ps_w_ctx.close()
        ps_e_ctx = ExitStack()
        ps_e = ps_e_ctx.enter_context(tc.tile_pool(name="ps_e", bufs=1, space="PSUM"))

        # early pass-2 transposes through the freed warm bank, overlapping
        # the fixup/softmax window below (scheduler orders by deps)
        qc_tiles = {}

        def emit_tr(s, pool, bank_pool):
            qc = p_qc.tile([P, MB, P], F16, tag="qc", name="qc")
            tp = bank_pool.tile([P, MB, P], F32, tag="tpe", name="tpe")
            for m in range(MB):
                nc.tensor.matmul(
                    tp[:, m, :],
                    hi_res[:, s, m * P : (m + 1) * P],
                    ident16[:],
                    start=True,
                    stop=True,
                )
            if s % 2 == 0:
                nc.scalar.copy(qc[:], tp[:])
            else:
                nc.vector.tensor_copy(qc[:], tp[:])
            qc_tiles[s] = qc

        for s0 in range(min(n_early, nsub)):
            emit_tr(s0, p_qc, ps_e)

        # ---- fixup in PSUM: a_ps[m] <- B[m] + sum_mp T((ah+al)[mp, m]) ----
        ah = pers.tile([P, MB, C], F16)
        al = pers.tile([P, MB, C], F16)
        for m in range(MB):
            nc.scalar.copy(ah[:, m, :], a_ps[m][:])
            nc.vector.tensor_tensor(
                al[:, m, :], a_ps[m][:], ah[:, m, :], mybir.AluOpType.subtract
            )
        for m in range(MB):
            for mp in range(MB):
                nc.tensor.matmul(
                    a_ps[m][:, mp * P : (mp + 1) * P],
                    ah[:, mp, m * P : (m + 1) * P],
                    ident16[:],
                    start=False,
                    stop=False,
                    skip_group_check=True,
                )
                nc.tensor.matmul(
                    a_ps[m][:, mp * P : (mp + 1) * P],
                    al[:, mp, m * P : (m + 1) * P],
                    ident16[:],
                    start=False,
                    stop=(mp == MB - 1),
                    skip_group_check=True,
                )

        # ---- softmax from PSUM: mfull = 2*gamma*softmax(-S) + 2*I (fp16) ----
        idg = pers.tile([P, P], F16)
        nc.scalar.mul(idg[:], ident16[:], 2.0)
        mfull = pers.tile([P, MB, C], F16)
        for m in range(MB):
            mn = p_sm.tile([P, 1], F32, tag="mn", name="mn")
            nc.vector.tensor_reduce(
                mn[:], a_ps[m][:], axis=mybir.AxisListType.X, op=mybir.AluOpType.min
            )
            mn2 = p_sm.tile([P, 1], F32, tag="mn2", name="mn2")
            nc.vector.tensor_scalar_mul(mn2[:], mn[:], 2.0)
            e = p_sm.tile([P, C], F32, tag="e", name="e")
            z = p_sm.tile([P, 1], F32, tag="z", name="z")
            nc.scalar.activation(
                e[:],
                a_ps[m][:],
                mybir.ActivationFunctionType.Exp,
                bias=mn2[:],
                scale=-2.0,
                accum_out=z[:],
            )
            rz = p_sm.tile([P, 1], F32, tag="rz", name="rz")
            nc.vector.reciprocal(rz[:], z[:])
            rzg = p_sm.tile([P, 1], F32, tag="rzg", name="rzg")
            nc.vector.tensor_mul(rzg[:], rz[:], gamma_sb[:])
            if m % 2 == 0:
                nc.scalar.mul(mfull[:, m, :], e[:], rzg[:])
            else:
                nc.vector.tensor_scalar_mul(mfull[:, m, :], e[:], rzg[:])
            nc.vector.tensor_tensor(
                mfull[:, m, m * P : (m + 1) * P],
                mfull[:, m, m * P : (m + 1) * P],
                idg[:],
                mybir.AluOpType.add,
            )
        ps_e_ctx.close()
        ps_a_ctx.close()
        ps_t4 = ctx.enter_context(
            tc.tile_pool(name="ps_t4", bufs=ps_t4_bufs, space="PSUM")
        )

        # ---- pass 2: PE chunk-transpose + value matmuls ----
        for s0 in range(n_early, min(tr_look, nsub)):
            emit_tr(s0, p_qc, ps_t4)
        for k in range(nk):
            of = p_out.tile([P, 2, 2, C], F32, tag="of", name="of")
            for u in range(2):
                for j in range(2):
                    s = 4 * k + 2 * u + j
                    if s + tr_look < nsub:
                        emit_tr(s + tr_look, p_qc, ps_t4)
                    vp = ps_v.tile([P, C], F32, tag="vp", name="vp")
                    qc = qc_tiles.pop(s)
                    for m in range(MB):
                        nc.tensor.matmul(
                            vp[:],
                            qc[:, m, :],
                            mfull[:, m, :],
                            start=(m == 0),
                            stop=(m == MB - 1),
                        )
                    if s % 2 == 1:
                        nc.scalar.copy(of[:, u, j, :], vp[:])
                    else:
                        nc.vector.tensor_copy(of[:, u, j, :], vp[:])
            if k >= nk - 2:
                for u in range(2):
                    nc.sync.dma_start(ow[k][:, u, :, :], of[:, u, :, :])
            else:
                nc.sync.dma_start(ow[k], of[:])

    nc.compile()
    return nc



def build_v6(
    n_rows=N_ROWS_FULL,
    tr_look=12,
    in_bufs=6,
    h_bufs=3,
    nwarm=14,
    split0=True,
    ps_t4_bufs=3,
    ps_v_bufs=3,
):
    """v6 = v5 (B = h2^T r, S = 2(B + B^T)) with:
      - in_bufs=6: pass-1 pace was gated by the xf recycle loop
        (DMA -> h2 -> r -> next DMA issue, ~4.7us over effective depth 2)
      - S/2 assembled fully in PSUM: ah/al are cast straight from the B
        banks, the 32 block-transpose matmuls accumulate back INTO the B
        banks (start=False continues the group), softmax reads PSUM with
        the x2 folded into the exp (scale=-2, bias=2*rowmin).  Deletes the
        4 s_sb copies and 16 DVE adds from the transition critical path.
      - softmax e->mfull scale moved to ScalarE (ACTIVATE scale=rzg AP).
    """
    assert n_rows % 512 == 0
    nsup = n_rows // 256
    nsub = n_rows // 128

    nc = bacc.Bacc(trn_type="TRN2", name="chanattn6")
    x = nc.dram_tensor("x", [n_rows, C], F32, kind="ExternalInput")
    gb = nc.dram_tensor("gammab", [P, 1], F32, kind="ExternalInput")  # = 2*gamma
    idm = nc.dram_tensor("ident", [P, P], F16, kind="ExternalInput")
    out = nc.dram_tensor("out", [n_rows, C], F32, kind="ExternalOutput")

    xv = x[:].rearrange("(i p j) c -> i p j c", p=P, j=2)
    ov = out[:].rearrange("(i p j) c -> i p j c", p=P, j=2)

    with tile.TileContext(nc) as tc, ExitStack() as ctx:
        pers = ctx.enter_context(tc.tile_pool(name="pers", bufs=1))
        p_in = ctx.enter_context(tc.tile_pool(name="p_in", bufs=in_bufs))
        p_h = ctx.enter_context(tc.tile_pool(name="p_h", bufs=h_bufs))
        p_sm = ctx.enter_context(tc.tile_pool(name="p_sm", bufs=2))
        p_qc = ctx.enter_context(tc.tile_pool(name="p_qc", bufs=tr_look))
        p_out = ctx.enter_context(tc.tile_pool(name="p_out", bufs=2))
        ps_a_ctx = ExitStack()
        ps_a = ps_a_ctx.enter_context(tc.tile_pool(name="ps_a", bufs=1, space="PSUM"))
        ps_w = ps_a_ctx.enter_context(tc.tile_pool(name="ps_w", bufs=1, space="PSUM"))

        hi_res = pers.tile([P, nsub, C], F16)  # holds h2 = 0.5*h
        ident16 = pers.tile([P, P], F16)
        nc.sync.dma_start(ident16[:], idm[:])
        gamma_sb = pers.tile([P, 1], F32)
        nc.sync.dma_start(gamma_sb[:], gb[:])

        if nwarm:
            wt = pers.tile([P, C], F16)
            nc.vector.memset(wt[:], 0)
            wps = ps_w.tile([P, C], F32, tag="wps", name="wps")
            for _ in range(nwarm):
                nc.tensor.matmul(
                    wps[:], wt[:, :P], wt[:], start=True, stop=True,
                    skip_group_check=True,
                )

        a_ps = [ps_a.tile([P, C], F32, tag=f"a{m}", name=f"a{m}") for m in range(MB)]

        # ---- pass 1: B = h2^T r ----
        for i in range(nsup):
            xf = p_in.tile([P, 2, C], F32, tag="xf", name="xf")
            if i == 0 and split0:
                nc.sync.dma_start(xf[:, 0, :], xv[0][:, 0, :])
                nc.sync.dma_start(xf[:, 1, :], xv[0][:, 1, :])
            else:
                nc.sync.dma_start(xf[:], xv[i])
            r = p_h.tile([P, 2, C], F16, tag="r", name="r")
            for j in range(2):
                s = 2 * i + j
                h2j = hi_res[:, s, :]
                nc.scalar.mul(h2j, xf[:, j, :], 0.5)
                nc.vector.tensor_tensor(
                    r[:, j, :], xf[:, j, :], h2j, mybir.AluOpType.subtract
                )
                for m in range(MB):
                    nc.tensor.matmul(
                        a_ps[m][:],
                        hi_res[:, s, m * P : (m + 1) * P],
                        r[:, j, :],
                        start=(s == 0),
                        stop=(s == nsub - 1),
                        skip_group_check=True,
                    )

        # ---- fixup in PSUM: a_ps[m] <- B[m] + sum_mp T((ah+al)[mp, m]) ----
        ah = pers.tile([P, MB, C], F16)
        al = pers.tile([P, MB, C], F16)
        for m in range(MB):
            nc.scalar.copy(ah[:, m, :], a_ps[m][:])
            nc.vector.tensor_tensor(
                al[:, m, :], a_ps[m][:], ah[:, m, :], mybir.AluOpType.subtract
            )
        for m in range(MB):
            for mp in range(MB):
                nc.tensor.matmul(
                    a_ps[m][:, mp * P : (mp + 1) * P],
                    ah[:, mp, m * P : (m + 1) * P],
                    ident16[:],
                    start=False,
                    stop=False,
                    skip_group_check=True,
                )
                nc.tensor.matmul(
                    a_ps[m][:, mp * P : (mp + 1) * P],
                    al[:, mp, m * P : (m + 1) * P],
                    ident16[:],
                    start=False,
                    stop=(mp == MB - 1),
                    skip_group_check=True,
                )

        # ---- softmax from PSUM: mfull = 2*gamma*softmax(-S) + 2*I (fp16) ----
        idg = pers.tile([P, P], F16)
        nc.scalar.mul(idg[:], ident16[:], 2.0)
        mfull = pers.tile([P, MB, C], F16)
        for m in range(MB):
            mn = p_sm.tile([P, 1], F32, tag="mn", name="mn")
            nc.vector.tensor_reduce(
                mn[:], a_ps[m][:], axis=mybir.AxisListType.X, op=mybir.AluOpType.min
            )
            mn2 = p_sm.tile([P, 1], F32, tag="mn2", name="mn2")
            nc.vector.tensor_scalar_mul(mn2[:], mn[:], 2.0)
            e = p_sm.tile([P, C], F32, tag="e", name="e")
            z = p_sm.tile([P, 1], F32, tag="z", name="z")
            nc.scalar.activation(
                e[:],
                a_ps[m][:],
                mybir.ActivationFunctionType.Exp,
                bias=mn2[:],
                scale=-2.0,
                accum_out=z[:],
            )
            rz = p_sm.tile([P, 1], F32, tag="rz", name="rz")
            nc.vector.reciprocal(rz[:], z[:])
            rzg = p_sm.tile([P, 1], F32, tag="rzg", name="rzg")
            nc.vector.tensor_mul(rzg[:], rz[:], gamma_sb[:])
            nc.scalar.mul(mfull[:, m, :], e[:], rzg[:])
            nc.vector.tensor_tensor(
                mfull[:, m, m * P : (m + 1) * P],
                mfull[:, m, m * P : (m + 1) * P],
                idg[:],
                mybir.AluOpType.add,
            )
        ps_a_ctx.close()
        ps_t4 = ctx.enter_context(
            tc.tile_pool(name="ps_t4", bufs=ps_t4_bufs, space="PSUM")
        )
        ps_v = ctx.enter_context(tc.tile_pool(name="ps_v", bufs=ps_v_bufs, space="PSUM"))

        # ---- pass 2: PE chunk-transpose + value matmuls ----
        qc_tiles = {}

        def emit_tr(s):
            qc = p_qc.tile([P, MB, P], F16, tag="qc", name="qc")
            tp = ps_t4.tile([P, MB, P], F32, tag="tp4", name="tp4")
            for m in range(MB):
                nc.tensor.matmul(
                    tp[:, m, :],
                    hi_res[:, s, m * P : (m + 1) * P],
                    ident16[:],
                    start=True,
                    stop=True,
                )
            if s % 2 == 0:
                nc.scalar.copy(qc[:], tp[:])
            else:
                nc.vector.tensor_copy(qc[:], tp[:])
            qc_tiles[s] = qc

        for s0 in range(min(tr_look, nsub)):
            emit_tr(s0)
        for i in range(nsup):
            of = p_out.tile([P, 2, C], F32, tag="of", name="of")
            for j in range(2):
                s = 2 * i + j
                if s + tr_look < nsub:
                    emit_tr(s + tr_look)
                vp = ps_v.tile([P, C], F32, tag="vp", name="vp")
                qc = qc_tiles.pop(s)
                for m in range(MB):
                    nc.tensor.matmul(
                        vp[:],
                        qc[:, m, :],
                        mfull[:, m, :],
                        start=(m == 0),
                        stop=(m == MB - 1),
                    )
                if s % 2 == 1:
                    nc.scalar.copy(of[:, j, :], vp[:])
                else:
                    nc.vector.tensor_copy(of[:, j, :], vp[:])
            nc.sync.dma_start(ov[i], of[:])

    nc.compile()
    return nc


DEFAULT_MODE = "v8"
GAMMA_MULT = {"v5": 2.0, "v6": 2.0, "v7": 2.0, "v8": 2.0}


def make_in_map(x_sample, gamma):
    return {
        "x": np.ascontiguousarray(x_sample, dtype=np.float32),
        "gammab": np.full(
            (P, 1), gamma * GAMMA_MULT.get(DEFAULT_MODE, 1.0), dtype=np.float32
        ),
        "ident": np.eye(P, dtype=np.float16),
    }


_NC_CACHE = {}


def _get_nc(n_rows=N_ROWS_FULL, s_mode=None):
    if s_mode is None:
        s_mode = DEFAULT_MODE
    key = (n_rows, s_mode)
    if key not in _NC_CACHE:
        if s_mode.startswith("v8"):
            _NC_CACHE[key] = build_v8(n_rows)
        elif s_mode.startswith("v7"):
            _NC_CACHE[key] = build_v7(n_rows)
        elif s_mode.startswith("v6"):
            _NC_CACHE[key] = build_v6(n_rows)
        elif s_mode.startswith("v5"):
            _NC_CACHE[key] = build_v5(n_rows)
        elif s_mode.startswith("v4"):
            lo_eng = "gps" if s_mode == "v4gps" else "dve"
            _NC_CACHE[key] = build_v4(n_rows, lo_eng=lo_eng)
        elif s_mode.startswith("v3"):
            lo_eng = "gps" if s_mode == "v3gps" else "dve"
            _NC_CACHE[key] = build_v3(n_rows, lo_eng=lo_eng)
        else:
            _NC_CACHE[key] = build(n_rows, s_mode)
    return _NC_CACHE[key]


def kernel(inputs, gamma):
    from concourse.bass_utils import run_bass_kernel_spmd

    x = np.asarray(inputs, dtype=np.float32)
    g = float(np.asarray(gamma, dtype=np.float32))
    assert x.shape == (B, H, W, C), x.shape

    nc = _get_nc()
    in_maps = [make_in_map(x[b].reshape(N_ROWS_FULL, C), g) for b in range(B)]
    res = run_bass_kernel_spmd(nc, in_maps, core_ids=list(range(N_CORES)))
    out = np.stack([r["out"] for r in res.results], axis=0)
    return out.reshape(B, H, W, C).astype(np.float32)

